# revision 20
# baseline (speedup 1.0000x reference)
"""Causal attention (N=4096, D=1024) + dropout, distributed over 8 TRN2 NeuronCores.

Sharding: sequence-parallel over Q rows, strided assignment (core c owns rows
c::8) so every core has an identical causal-work profile (SPMD: one program).
K/V projections are sharded: core c computes K/V for contiguous key block
[512c, 512c+512), then one AllGather shares all K/V shards with every core.

Per-core device program (all matmuls bf16 inputs, f32 PSUM accumulation):
  qT = wqT.T @ xqT            [d_out, 512]   (feature-major)
  kT_own = wkT.T @ xkvT       [d_out, 512]   (own key block)
  v_own  = xkvT.T @ wvT       [512, d_out]
  AllGather(kT_own | v_own)  -> kT [d_out, 4096], v [4096, d_out]
  per qi-block t (128 local rows = global span 1024t..1024(t+1)):
    sT[kj,qi] = kT.T @ qT     only kj < 1024(t+1)  (causal block skipping)
    p = exp(sT/32) * causal_edge_mask
    denom[qi] = ones-matmul over p
    pm = p * dropout_mask
    out[qi,:] = (pm.T @ v) * 1/(0.9*denom)
"""

import numpy as np
import ml_dtypes

import concourse.bass as bass
import concourse.mybir as mybir
import concourse.tile as tile
from concourse import bacc
from concourse.bass_utils import run_bass_kernel_spmd

N, D, P, NCORES = 4096, 1024, 128, 8
R = N // NCORES          # 512 q rows per core
DH = D // P              # 8 chunks of the feature dim
NT = R // P              # 4 qi-blocks per core
KEEP = 0.9

bf = mybir.dt.bfloat16
f32 = mybir.dt.float32
bfnp = ml_dtypes.bfloat16


NLOC = 2                 # 512-key chunks recomputed locally during the all-gather
SHK = N - 512 * NLOC     # keys shared via all-gather
OWN = SHK // NCORES      # 384 keys projected per core


def _body(nc, tc, aps):
    xqt, xkvt, xlt, wqt, wkt, wvt, cm, dm, out = aps
    xqt3 = xqt.rearrange("(h p) r -> p h r", p=P)
    xkvt3 = xkvt.rearrange("(h p) r -> p h r", p=P)
    xlt3 = xlt.rearrange("(h p) r -> p h r", p=P)
    w3 = {k: v.rearrange("(h p) o -> p h o", p=P) for k, v in
          dict(q=wqt, k=wkt, v=wvt).items()}
    cm3 = cm.rearrange("(e p) i -> p e i", p=P)
    dm3 = dm.rearrange("(kc p) q -> p kc q", p=P)

    import contextlib
    with contextlib.ExitStack() as ctx:
        big = ctx.enter_context(tc.tile_pool(name="big", bufs=1))
        dram = ctx.enter_context(tc.tile_pool(name="dram", bufs=1, space="DRAM"))
        psA = ctx.enter_context(tc.tile_pool(name="psA", bufs=4, space="PSUM"))
        psO = ctx.enter_context(tc.tile_pool(name="psO", bufs=3, space="PSUM"))
        psD = ctx.enter_context(tc.tile_pool(name="psD", bufs=1, space="PSUM"))

        # persistent SBUF: qT, local K/V chunks (keys < 1024), gathered sections
        qT = big.tile([P, DH, R], bf, tag="qT")
        kT_t = [big.tile([P, DH, 512], bf, tag=f"kT{rc}", name=f"kT{rc}")
                for rc in range(NLOC)]
        v_t = [big.tile([P, 4, D], bf, tag=f"v{rc}", name=f"v{rc}")
               for rc in range(NLOC)]
        kTg = [big.tile([P, DH, OWN], bf, tag=f"kTg{s}", name=f"kTg{s}")
               for s in range(NCORES)]
        vg = [big.tile([P, OWN // P, D], bf, tag=f"vg{s}", name=f"vg{s}")
              for s in range(NCORES)]

        # DRAM bounce buffers for the K/V all-gather
        send = dram.tile([2, D * OWN], bf, name="send")
        gath = dram.tile([NCORES, 2, D * OWN], bf, addr_space="Shared",
                         name="gath")
        send_k = send[0].rearrange("(h p n) -> p h n", p=P, h=DH)
        send_v = send[1].rearrange("(n2 p d) -> p n2 d", p=P, n2=OWN // P)

        def load_w(pool, which, nm, step=4):
            w_s = pool.tile([P, DH, D], bf, tag="w", name=nm, bufs=2)
            for h in range(0, DH, step):
                nc.sync.dma_start(w_s[:, h:h + step, :],
                                  w3[which][:, h:h + step, :])
            return w_s

        def load_x(pool, src3, nm, tag="xs", w=R, step=4):
            x_s = pool.tile([P, DH, w], bf, tag=tag, name=nm, bufs=2,
                            padded_shape=[P, DH, R])
            for h in range(0, DH, step):
                nc.sync.dma_start(x_s[:, h:h + step, :],
                                  src3[:, h:h + step, :])
            return x_s

        def proj_kt(dst, w_s, x_s, snd=None):
            nk = x_s.shape[-1]
            for oh in range(DH):
                pk = psA.tile([P, 512], f32, tag="mm", name="pk")
                for di in range(DH):
                    nc.tensor.matmul(pk[:, :nk], w_s[:, di, oh * P:(oh + 1) * P],
                                     x_s[:, di, :],
                                     start=(di == 0), stop=(di == DH - 1))
                nc.scalar.copy(dst[:, oh, :], pk[:, :nk])
                if snd is not None:
                    nc.sync.dma_start(snd[:, oh, :], dst[:, oh, :])

        def proj_v(dst, w_s, x_s, snd=None):
            nk = x_s.shape[-1]
            for rr in range(nk // P):
                for dvh in range(2):
                    pv = psA.tile([P, 512], f32, tag="mm", name="pv")
                    for di in range(DH):
                        nc.tensor.matmul(pv[:], x_s[:, di, rr * P:(rr + 1) * P],
                                         w_s[:, di, dvh * 512:(dvh + 1) * 512],
                                         start=(di == 0), stop=(di == DH - 1))
                    nc.vector.tensor_copy(dst[:, rr, dvh * 512:(dvh + 1) * 512],
                                          pv[:])
                if snd is not None:
                    nc.sync.dma_start(snd[:, rr, :], dst[:, rr, :])

        with tc.tile_pool(name="wp", bufs=1) as wp:
            # ---- K/V projection of own key block, then all-gather ----
            wk_s = load_w(wp, "k", "wk_s", step=1)
            xkv_s = load_x(wp, xkvt3, "xkv_s", w=OWN, step=2)
            kT_own = wp.tile([P, DH, OWN], bf, tag="kT_own")
            proj_kt(kT_own, wk_s, xkv_s, send_k)
            wv_s = load_w(wp, "v", "wv_s")
            v_own = wp.tile([P, OWN // P, D], bf, tag="v_own")
            proj_v(v_own, wv_s, xkv_s, send_v)

            nc.gpsimd.collective_compute(
                "AllGather", mybir.AluOpType.bypass,
                replica_groups=[list(range(NCORES))],
                ins=[send.opt()], outs=[gath.opt()])

            # ---- q projection (overlaps the all-gather) ----
            wq_s = load_w(wp, "q", "wq_s")
            xq_s = load_x(wp, xqt3, "xq_s")
            for oh in range(DH):
                pq = psA.tile([P, 512], f32, tag="mm", name="pq")
                for di in range(DH):
                    nc.tensor.matmul(pq[:], wq_s[:, di, oh * P:(oh + 1) * P],
                                     xq_s[:, di, :],
                                     start=(di == 0), stop=(di == DH - 1))
                nc.scalar.copy(qT[:, oh, :], pq[:])

            # ---- local recompute of early K/V chunks (fills the AG window) ----
            wk2 = load_w(wp, "k", "wk2")
            wv2 = load_w(wp, "v", "wv2")
            for rc in range(NLOC):
                xl_s = load_x(wp, xlt3[:, :, rc * 512:(rc + 1) * 512],
                              f"xl{rc}")
                proj_kt(kT_t[rc], wk2, xl_s)
                proj_v(v_t[rc], wv2, xl_s)

        # unpack gathered K/V sections into SBUF
        for s in range(NCORES):
            g_k = gath[s, 0].rearrange("(h p n) -> p h n", p=P, h=DH)
            g_v = gath[s, 1].rearrange("(n2 p d) -> p n2 d", p=P, n2=OWN // P)
            nc.gpsimd.dma_start(kTg[s][:], g_k[:])
            nc.gpsimd.dma_start(vg[s][:], g_v[:])

        def kt_sl(kj, dh):
            if kj < 4 * NLOC:
                return kT_t[kj // 4][:, dh, (kj % 4) * P:(kj % 4 + 1) * P]
            s = (P * kj - 512 * NLOC) // OWN
            col = P * kj - 512 * NLOC - OWN * s
            return kTg[s][:, dh, col:col + P]

        def v_sl(kj, dvh):
            if kj < 4 * NLOC:
                return v_t[kj // 4][:, kj % 4, dvh * 512:(dvh + 1) * 512]
            s = (P * kj - 512 * NLOC) // OWN
            rr = (P * kj - 512 * NLOC - OWN * s) // P
            return vg[s][:, rr, dvh * 512:(dvh + 1) * 512]

        # ---- attention ----
        with tc.tile_pool(name="at", bufs=4) as at, \
             tc.tile_pool(name="cs", bufs=1) as cs, \
             tc.tile_pool(name="dmp", bufs=4) as dmp:
            ones = cs.tile([P, 1], bf, tag="ones")
            nc.vector.memset(ones[:], 1.0)
            cm_s = cs.tile([P, 8, P], bf, tag="cm")
            nc.sync.dma_start(cm_s[:], cm3[:])

            for t in range(NT):
                nkj = 8 * (t + 1)           # 128-wide key chunks this block
                ng = nkj // 4               # psum groups of 4 chunks
                qsl = slice(t * P, (t + 1) * P)
                dmt = dmp.tile([P, 32, P], bf, tag="dm")
                half = max(nkj // 2, 4)
                for kc in range(0, nkj, half):
                    nc.scalar.dma_start(dmt[:, kc:kc + half, :],
                                        dm3[:, kc:kc + half, qsl])

                po = [psO.tile([P, 512], f32, tag="po", name=f"po{t}_{i}")
                      for i in range(2)]
                pd = psD.tile([P, 1], f32, tag="pd")

                for g in range(ng):
                    ps = psA.tile([P, 4, P], f32, tag="mm", name="ps")
                    for kj4 in range(4):
                        kj = g * 4 + kj4
                        for dh in range(DH):
                            nc.tensor.matmul(ps[:, kj4, :],
                                             kt_sl(kj, dh),
                                             qT[:, dh, qsl],
                                             start=(dh == 0), stop=(dh == DH - 1))
                    p = at.tile([P, 4, P], bf, tag="p")
                    nc.scalar.activation(p[:], ps[:],
                                         mybir.ActivationFunctionType.Exp,
                                         scale=1.0 / 32.0)
                    if g >= 2 * t:          # causal edge band: zero j > row
                        e = g - 2 * t
                        nc.vector.tensor_mul(p[:], p[:], cm_s[:, 4 * e:4 * e + 4, :])
                    pm = at.tile([P, 4, P], bf, tag="pm")
                    nc.vector.tensor_mul(pm[:], p[:], dmt[:, 4 * g:4 * g + 4, :])
                    for kj4 in range(4):
                        kj = g * 4 + kj4
                        nc.tensor.matmul(pd[:], p[:, kj4, :], ones[:],
                                         start=(kj == 0), stop=(kj == nkj - 1))
                        for dvh in range(2):
                            nc.tensor.matmul(po[dvh][:], pm[:, kj4, :],
                                             v_sl(kj, dvh),
                                             start=(kj == 0), stop=(kj == nkj - 1))

                den = at.tile([P, 1], f32, tag="den")
                nc.vector.tensor_scalar_mul(den[:], pd[:], KEEP)
                rec = at.tile([P, 1], f32, tag="rec")
                nc.vector.reciprocal(rec[:], den[:])
                for dvh in range(2):
                    ob = at.tile([P, 512], f32, tag="ob")
                    nc.vector.tensor_scalar_mul(ob[:], po[dvh][:], rec[:])
                    nc.sync.dma_start(out[qsl, dvh * 512:(dvh + 1) * 512], ob[:])


_CACHE = {}


def _get_nc():
    if "nc" not in _CACHE:
        nc = bacc.Bacc("TRN2", target_bir_lowering=False, debug=False,
                       num_devices=NCORES)
        aps = (
            nc.dram_tensor("xqt", [D, R], bf, kind="ExternalInput").ap(),
            nc.dram_tensor("xkvt", [D, OWN], bf, kind="ExternalInput").ap(),
            nc.dram_tensor("xlt", [D, 512 * NLOC], bf,
                           kind="ExternalInput").ap(),
            nc.dram_tensor("wqt", [D, D], bf, kind="ExternalInput").ap(),
            nc.dram_tensor("wkt", [D, D], bf, kind="ExternalInput").ap(),
            nc.dram_tensor("wvt", [D, D], bf, kind="ExternalInput").ap(),
            nc.dram_tensor("cm", [1024, P], bf, kind="ExternalInput").ap(),
            nc.dram_tensor("dm", [N, R], bf, kind="ExternalInput").ap(),
            nc.dram_tensor("out", [R, D], f32, kind="ExternalOutput").ap(),
        )
        with tile.TileContext(nc) as tc:
            _body(nc, tc, aps)
        nc.compile()
        _CACHE["nc"] = nc
    return _CACHE["nc"]


def _dropout_keep():
    # Must reproduce the reference's mask bit-exactly. The reference calls
    # jax.random.bernoulli on whatever jax backend is the process default
    # (different backends produce different threefry layouts), so mirror the
    # call verbatim with no device pinning.
    if "keep" not in _CACHE:
        import jax
        keep = np.asarray(
            jax.random.bernoulli(jax.random.key(42), KEEP, (N, N)))
        _CACHE["keep"] = keep
    return _CACHE["keep"]


def _c(a):
    return np.ascontiguousarray(a)


def kernel(x, wq, wk, wv, **_spmd_kwargs):
    x = np.asarray(x, np.float32)
    wqt = _c(np.asarray(wq, np.float32).astype(bfnp).T)
    wkt = _c(np.asarray(wk, np.float32).astype(bfnp).T)
    wvt = _c(np.asarray(wv, np.float32).astype(bfnp).T)
    xb = x.astype(bfnp)
    xlt = _c(xb[:512 * NLOC].T)
    keep = _dropout_keep()

    jj = np.arange(1024)[:, None]
    ii = np.arange(P)[None, :]

    in_maps = []
    for c in range(NCORES):
        cm = ((jj <= 8 * ii + c).astype(np.float32)).astype(bfnp)
        dm = _c(keep[c::NCORES, :].T.astype(bfnp))
        in_maps.append({
            "xqt": _c(xb[c::NCORES].T),
            "xkvt": _c(xb[512 * NLOC + c * OWN:512 * NLOC + (c + 1) * OWN].T),
            "xlt": xlt,
            "wqt": wqt, "wkt": wkt, "wvt": wvt,
            "cm": _c(cm), "dm": dm,
        })

    nc = _get_nc()
    res = run_bass_kernel_spmd(nc, in_maps, core_ids=list(range(NCORES)),
                               **_spmd_kwargs)
    out = np.empty((N, D), np.float32)
    for c in range(NCORES):
        out[c::NCORES] = res.results[c]["out"]
    if _spmd_kwargs:
        kernel.last_result = res
    return out


# revision 21
# speedup vs baseline: 1.2993x; 1.2993x over previous
"""Causal attention (N=4096, D=1024) + dropout, distributed over 8 TRN2 NeuronCores.

Sharding: sequence-parallel over Q rows, strided assignment (core c owns rows
c::8) so every core has an identical causal-work profile (SPMD: one program).
K/V projections are sharded: core c computes K/V for contiguous key block
[512c, 512c+512), then one AllGather shares all K/V shards with every core.

Per-core device program (all matmuls bf16 inputs, f32 PSUM accumulation):
  qT = wqT.T @ xqT            [d_out, 512]   (feature-major)
  kT_own = wkT.T @ xkvT       [d_out, 512]   (own key block)
  v_own  = xkvT.T @ wvT       [512, d_out]
  AllGather(kT_own | v_own)  -> kT [d_out, 4096], v [4096, d_out]
  per qi-block t (128 local rows = global span 1024t..1024(t+1)):
    sT[kj,qi] = kT.T @ qT     only kj < 1024(t+1)  (causal block skipping)
    p = exp(sT/32) * causal_edge_mask
    denom[qi] = ones-matmul over p
    pm = p * dropout_mask
    out[qi,:] = (pm.T @ v) * 1/(0.9*denom)
"""

import numpy as np
import ml_dtypes

import concourse.bass as bass
import concourse.mybir as mybir
import concourse.tile as tile
from concourse import bacc
from concourse.bass_utils import run_bass_kernel_spmd

N, D, P, NCORES = 4096, 1024, 128, 8
R = N // NCORES          # 512 q rows per core
DH = D // P              # 8 chunks of the feature dim
NT = R // P              # 4 qi-blocks per core
KEEP = 0.9

bf = mybir.dt.bfloat16
f32 = mybir.dt.float32
bfnp = ml_dtypes.bfloat16


NLOC = 2                 # 512-key chunks recomputed locally during the all-gather
SHK = N - 512 * NLOC     # keys shared via all-gather
OWN = SHK // NCORES      # 384 keys projected per core


def _body(nc, tc, aps):
    xqt, xkvt, xlt, wqt, wkt, wvt, cm, dm, out = aps
    xqt3 = xqt.rearrange("(h p) r -> p h r", p=P)
    xkvt3 = xkvt.rearrange("(h p) r -> p h r", p=P)
    xlt3 = xlt.rearrange("(h p) r -> p h r", p=P)
    w3 = {k: v.rearrange("(h p) o -> p h o", p=P) for k, v in
          dict(q=wqt, k=wkt, v=wvt).items()}
    cm3 = cm.rearrange("(e p) i -> p e i", p=P)
    dm3 = dm.rearrange("(kc p) q -> p kc q", p=P)

    import contextlib
    with contextlib.ExitStack() as ctx:
        big = ctx.enter_context(tc.tile_pool(name="big", bufs=1))
        dram = ctx.enter_context(tc.tile_pool(name="dram", bufs=1, space="DRAM"))
        psA = ctx.enter_context(tc.tile_pool(name="psA", bufs=3, space="PSUM"))
        psO = ctx.enter_context(tc.tile_pool(name="psO", bufs=3, space="PSUM"))
        psD = ctx.enter_context(tc.tile_pool(name="psD", bufs=2, space="PSUM"))

        # persistent SBUF: qT, local K/V chunks (keys < 1024), gathered sections
        qT = big.tile([P, DH, R], bf, tag="qT")
        kT_t = [big.tile([P, DH, 512], bf, tag=f"kT{rc}", name=f"kT{rc}")
                for rc in range(NLOC)]
        v_t = [big.tile([P, 4, D], bf, tag=f"v{rc}", name=f"v{rc}")
               for rc in range(NLOC)]
        kTg = [big.tile([P, DH, OWN], bf, tag=f"kTg{s}", name=f"kTg{s}")
               for s in range(NCORES)]
        vg = [big.tile([P, OWN // P, D], bf, tag=f"vg{s}", name=f"vg{s}")
              for s in range(NCORES)]

        # DRAM bounce buffers for the K/V all-gather
        send = dram.tile([2, D * OWN], bf, name="send")
        gath = dram.tile([NCORES, 2, D * OWN], bf, addr_space="Shared",
                         name="gath")
        send_k = send[0].rearrange("(h p n) -> p h n", p=P, h=DH)
        send_v = send[1].rearrange("(n2 p d) -> p n2 d", p=P, n2=OWN // P)

        def load_w(pool, which, nm, step=4):
            w_s = pool.tile([P, DH, D], bf, tag="w", name=nm, bufs=2)
            for h in range(0, DH, step):
                nc.sync.dma_start(w_s[:, h:h + step, :],
                                  w3[which][:, h:h + step, :])
            return w_s

        def load_x(pool, src3, nm, tag="xs", w=R, step=4):
            x_s = pool.tile([P, DH, w], bf, tag=tag, name=nm, bufs=2,
                            padded_shape=[P, DH, R])
            for h in range(0, DH, step):
                nc.sync.dma_start(x_s[:, h:h + step, :],
                                  src3[:, h:h + step, :])
            return x_s

        def proj_kt(dst, w_s, x_s, snd=None):
            nk = x_s.shape[-1]
            for oh in range(DH):
                pk = psA.tile([P, 512], f32, tag="mm", name="pk")
                for di in range(DH):
                    nc.tensor.matmul(pk[:, :nk], w_s[:, di, oh * P:(oh + 1) * P],
                                     x_s[:, di, :],
                                     start=(di == 0), stop=(di == DH - 1))
                nc.scalar.copy(dst[:, oh, :], pk[:, :nk])
                if snd is not None:
                    nc.sync.dma_start(snd[:, oh, :], dst[:, oh, :])

        def proj_v(dst, w_s, x_s, snd=None):
            nk = x_s.shape[-1]
            for rr in range(nk // P):
                for dvh in range(2):
                    pv = psA.tile([P, 512], f32, tag="mm", name="pv")
                    for di in range(DH):
                        nc.tensor.matmul(pv[:], x_s[:, di, rr * P:(rr + 1) * P],
                                         w_s[:, di, dvh * 512:(dvh + 1) * 512],
                                         start=(di == 0), stop=(di == DH - 1))
                    nc.vector.tensor_copy(dst[:, rr, dvh * 512:(dvh + 1) * 512],
                                          pv[:])
                if snd is not None:
                    nc.sync.dma_start(snd[:, rr, :], dst[:, rr, :])

        with tc.tile_pool(name="wp", bufs=1) as wp:
            # ---- K/V projection of own key block, then all-gather ----
            wk_s = load_w(wp, "k", "wk_s", step=1)
            xkv_s = load_x(wp, xkvt3, "xkv_s", w=OWN, step=2)
            kT_own = wp.tile([P, DH, OWN], bf, tag="kT_own")
            proj_kt(kT_own, wk_s, xkv_s, send_k)
            wv_s = load_w(wp, "v", "wv_s")
            v_own = wp.tile([P, OWN // P, D], bf, tag="v_own")
            proj_v(v_own, wv_s, xkv_s, send_v)

            nc.gpsimd.collective_compute(
                "AllGather", mybir.AluOpType.bypass,
                replica_groups=[list(range(NCORES))],
                ins=[send.opt()], outs=[gath.opt()])

            # ---- q projection (overlaps the all-gather) ----
            wq_s = load_w(wp, "q", "wq_s")
            xq_s = load_x(wp, xqt3, "xq_s")
            for oh in range(DH):
                pq = psA.tile([P, 512], f32, tag="mm", name="pq")
                for di in range(DH):
                    nc.tensor.matmul(pq[:], wq_s[:, di, oh * P:(oh + 1) * P],
                                     xq_s[:, di, :],
                                     start=(di == 0), stop=(di == DH - 1))
                nc.scalar.copy(qT[:, oh, :], pq[:])

            # ---- local recompute of early K/V chunks (fills the AG window) ----
            wk2 = load_w(wp, "k", "wk2")
            wv2 = load_w(wp, "v", "wv2")
            for rc in range(NLOC):
                xl_s = load_x(wp, xlt3[:, :, rc * 512:(rc + 1) * 512],
                              f"xl{rc}")
                proj_kt(kT_t[rc], wk2, xl_s)
                proj_v(v_t[rc], wv2, xl_s)

        # unpack gathered K/V sections into SBUF
        for s in range(NCORES):
            g_k = gath[s, 0].rearrange("(h p n) -> p h n", p=P, h=DH)
            g_v = gath[s, 1].rearrange("(n2 p d) -> p n2 d", p=P, n2=OWN // P)
            nc.gpsimd.dma_start(kTg[s][:], g_k[:])
            nc.gpsimd.dma_start(vg[s][:], g_v[:])

        def kt_sl(kj, dh):
            if kj < 4 * NLOC:
                return kT_t[kj // 4][:, dh, (kj % 4) * P:(kj % 4 + 1) * P]
            s = (P * kj - 512 * NLOC) // OWN
            col = P * kj - 512 * NLOC - OWN * s
            return kTg[s][:, dh, col:col + P]

        def v_sl(kj, dvh):
            if kj < 4 * NLOC:
                return v_t[kj // 4][:, kj % 4, dvh * 512:(dvh + 1) * 512]
            s = (P * kj - 512 * NLOC) // OWN
            rr = (P * kj - 512 * NLOC - OWN * s) // P
            return vg[s][:, rr, dvh * 512:(dvh + 1) * 512]

        # ---- attention ----
        with tc.tile_pool(name="at", bufs=4) as at, \
             tc.tile_pool(name="cs", bufs=1) as cs, \
             tc.tile_pool(name="dmp", bufs=4) as dmp:
            ones = cs.tile([P, 1], bf, tag="ones")
            nc.vector.memset(ones[:], 1.0)
            cm_s = cs.tile([P, 8, P], bf, tag="cm")
            nc.sync.dma_start(cm_s[:], cm3[:])

            for t in range(NT):
                nkj = 8 * (t + 1)           # 128-wide key chunks this block
                ng = nkj // 4               # psum groups of 4 chunks
                qsl = slice(t * P, (t + 1) * P)
                dmt = dmp.tile([P, 32, P], bf, tag="dm")
                half = max(nkj // 2, 4)
                for kc in range(0, nkj, half):
                    nc.scalar.dma_start(dmt[:, kc:kc + half, :],
                                        dm3[:, kc:kc + half, qsl])

                po = [psO.tile([P, 512], f32, tag="po", name=f"po{t}_{i}")
                      for i in range(2)]
                pd = psD.tile([P, 1], f32, tag="pd")

                for g in range(ng):
                    ps = psA.tile([P, 4, P], f32, tag="mm", name="ps")
                    for kj4 in range(4):
                        kj = g * 4 + kj4
                        for dh in range(DH):
                            nc.tensor.matmul(ps[:, kj4, :],
                                             kt_sl(kj, dh),
                                             qT[:, dh, qsl],
                                             start=(dh == 0), stop=(dh == DH - 1))
                    p = at.tile([P, 4, P], bf, tag="p")
                    nc.scalar.activation(p[:], ps[:],
                                         mybir.ActivationFunctionType.Exp,
                                         scale=1.0 / 32.0)
                    if g >= 2 * t:          # causal edge band: zero j > row
                        e = g - 2 * t
                        nc.vector.tensor_mul(p[:], p[:], cm_s[:, 4 * e:4 * e + 4, :])
                    pm = at.tile([P, 4, P], bf, tag="pm")
                    nc.vector.tensor_mul(pm[:], p[:], dmt[:, 4 * g:4 * g + 4, :])
                    for kj4 in range(4):
                        kj = g * 4 + kj4
                        nc.tensor.matmul(pd[:], p[:, kj4, :], ones[:],
                                         start=(kj == 0), stop=(kj == nkj - 1))
                        for dvh in range(2):
                            nc.tensor.matmul(po[dvh][:], pm[:, kj4, :],
                                             v_sl(kj, dvh),
                                             start=(kj == 0), stop=(kj == nkj - 1))

                den = at.tile([P, 1], f32, tag="den")
                nc.vector.tensor_scalar_mul(den[:], pd[:], KEEP)
                rec = at.tile([P, 1], f32, tag="rec")
                nc.vector.reciprocal(rec[:], den[:])
                for dvh in range(2):
                    ob = at.tile([P, 512], f32, tag="ob")
                    nc.vector.tensor_scalar_mul(ob[:], po[dvh][:], rec[:])
                    nc.sync.dma_start(out[qsl, dvh * 512:(dvh + 1) * 512], ob[:])


_CACHE = {}


def _get_nc():
    if "nc" not in _CACHE:
        nc = bacc.Bacc("TRN2", target_bir_lowering=False, debug=False,
                       num_devices=NCORES)
        aps = (
            nc.dram_tensor("xqt", [D, R], bf, kind="ExternalInput").ap(),
            nc.dram_tensor("xkvt", [D, OWN], bf, kind="ExternalInput").ap(),
            nc.dram_tensor("xlt", [D, 512 * NLOC], bf,
                           kind="ExternalInput").ap(),
            nc.dram_tensor("wqt", [D, D], bf, kind="ExternalInput").ap(),
            nc.dram_tensor("wkt", [D, D], bf, kind="ExternalInput").ap(),
            nc.dram_tensor("wvt", [D, D], bf, kind="ExternalInput").ap(),
            nc.dram_tensor("cm", [1024, P], bf, kind="ExternalInput").ap(),
            nc.dram_tensor("dm", [N, R], bf, kind="ExternalInput").ap(),
            nc.dram_tensor("out", [R, D], f32, kind="ExternalOutput").ap(),
        )
        with tile.TileContext(nc) as tc:
            _body(nc, tc, aps)
        nc.compile()
        _CACHE["nc"] = nc
    return _CACHE["nc"]


def _dropout_keep():
    # Must reproduce the reference's mask bit-exactly. The reference calls
    # jax.random.bernoulli on whatever jax backend is the process default
    # (different backends produce different threefry layouts), so mirror the
    # call verbatim with no device pinning.
    if "keep" not in _CACHE:
        import jax
        keep = np.asarray(
            jax.random.bernoulli(jax.random.key(42), KEEP, (N, N)))
        _CACHE["keep"] = keep
    return _CACHE["keep"]


def _c(a):
    return np.ascontiguousarray(a)


def kernel(x, wq, wk, wv, **_spmd_kwargs):
    x = np.asarray(x, np.float32)
    wqt = _c(np.asarray(wq, np.float32).astype(bfnp).T)
    wkt = _c(np.asarray(wk, np.float32).astype(bfnp).T)
    wvt = _c(np.asarray(wv, np.float32).astype(bfnp).T)
    xb = x.astype(bfnp)
    xlt = _c(xb[:512 * NLOC].T)
    keep = _dropout_keep()

    jj = np.arange(1024)[:, None]
    ii = np.arange(P)[None, :]

    in_maps = []
    for c in range(NCORES):
        cm = ((jj <= 8 * ii + c).astype(np.float32)).astype(bfnp)
        dm = _c(keep[c::NCORES, :].T.astype(bfnp))
        in_maps.append({
            "xqt": _c(xb[c::NCORES].T),
            "xkvt": _c(xb[512 * NLOC + c * OWN:512 * NLOC + (c + 1) * OWN].T),
            "xlt": xlt,
            "wqt": wqt, "wkt": wkt, "wvt": wvt,
            "cm": _c(cm), "dm": dm,
        })

    nc = _get_nc()
    res = run_bass_kernel_spmd(nc, in_maps, core_ids=list(range(NCORES)),
                               **_spmd_kwargs)
    out = np.empty((N, D), np.float32)
    for c in range(NCORES):
        out[c::NCORES] = res.results[c]["out"]
    if _spmd_kwargs:
        kernel.last_result = res
    return out


# revision 22
# speedup vs baseline: 1.3519x; 1.0405x over previous
"""Causal attention (N=4096, D=1024) + dropout, distributed over 8 TRN2 NeuronCores.

Sharding: sequence-parallel over Q rows, strided assignment (core c owns rows
c::8) so every core has an identical causal-work profile (SPMD: one program).
K/V projections are sharded: core c computes K/V for contiguous key block
[512c, 512c+512), then one AllGather shares all K/V shards with every core.

Per-core device program (all matmuls bf16 inputs, f32 PSUM accumulation):
  qT = wqT.T @ xqT            [d_out, 512]   (feature-major)
  kT_own = wkT.T @ xkvT       [d_out, 512]   (own key block)
  v_own  = xkvT.T @ wvT       [512, d_out]
  AllGather(kT_own | v_own)  -> kT [d_out, 4096], v [4096, d_out]
  per qi-block t (128 local rows = global span 1024t..1024(t+1)):
    sT[kj,qi] = kT.T @ qT     only kj < 1024(t+1)  (causal block skipping)
    p = exp(sT/32) * causal_edge_mask
    denom[qi] = ones-matmul over p
    pm = p * dropout_mask
    out[qi,:] = (pm.T @ v) * 1/(0.9*denom)
"""

import numpy as np
import ml_dtypes

import concourse.bass as bass
import concourse.mybir as mybir
import concourse.tile as tile
from concourse import bacc
from concourse.bass_utils import run_bass_kernel_spmd

N, D, P, NCORES = 4096, 1024, 128, 8
R = N // NCORES          # 512 q rows per core
DH = D // P              # 8 chunks of the feature dim
NT = R // P              # 4 qi-blocks per core
KEEP = 0.9

bf = mybir.dt.bfloat16
f32 = mybir.dt.float32
bfnp = ml_dtypes.bfloat16


NLOC = 2                 # 512-key chunks recomputed locally during the all-gather
SHK = N - 512 * NLOC     # keys shared via all-gather
OWN = SHK // NCORES      # 384 keys projected per core


def _body(nc, tc, aps):
    xqt, xkvt, xlt, wqt, wkt, wvt, cm, dm, out = aps
    xqt3 = xqt.rearrange("(h p) r -> p h r", p=P)
    xkvt3 = xkvt.rearrange("(h p) r -> p h r", p=P)
    xlt3 = xlt.rearrange("(h p) r -> p h r", p=P)
    w3 = {k: v.rearrange("(h p) o -> p h o", p=P) for k, v in
          dict(q=wqt, k=wkt, v=wvt).items()}
    cm3 = cm.rearrange("(e p) i -> p e i", p=P)
    dm3 = dm.rearrange("(kc p) q -> p kc q", p=P)

    import contextlib
    with contextlib.ExitStack() as ctx:
        big = ctx.enter_context(tc.tile_pool(name="big", bufs=1))
        dram = ctx.enter_context(tc.tile_pool(name="dram", bufs=1, space="DRAM"))
        psA = ctx.enter_context(tc.tile_pool(name="psA", bufs=4, space="PSUM"))
        psO = ctx.enter_context(tc.tile_pool(name="psO", bufs=2, space="PSUM"))
        psD = ctx.enter_context(tc.tile_pool(name="psD", bufs=2, space="PSUM"))

        # persistent SBUF: qT, local K/V chunks (keys < 1024), gathered sections
        qT = big.tile([P, DH, R], bf, tag="qT")
        kT_t = [big.tile([P, DH, 512], bf, tag=f"kT{rc}", name=f"kT{rc}")
                for rc in range(NLOC)]
        v_t = [big.tile([P, 4, D], bf, tag=f"v{rc}", name=f"v{rc}")
               for rc in range(NLOC)]
        kTg = [big.tile([P, DH, OWN], bf, tag=f"kTg{s}", name=f"kTg{s}")
               for s in range(NCORES)]
        vg = [big.tile([P, OWN // P, D], bf, tag=f"vg{s}", name=f"vg{s}")
              for s in range(NCORES)]

        # DRAM bounce buffers for the K/V all-gather
        send = dram.tile([2, D * OWN], bf, name="send")
        gath = dram.tile([NCORES, 2, D * OWN], bf, addr_space="Shared",
                         name="gath")
        send_k = send[0].rearrange("(h p n) -> p h n", p=P, h=DH)
        send_v = send[1].rearrange("(n2 p d) -> p n2 d", p=P, n2=OWN // P)

        def load_w(pool, which, nm, step=4):
            w_s = pool.tile([P, DH, D], bf, tag="w", name=nm, bufs=2)
            for h in range(0, DH, step):
                nc.sync.dma_start(w_s[:, h:h + step, :],
                                  w3[which][:, h:h + step, :])
            return w_s

        def load_x(pool, src3, nm, tag="xs", w=R, step=4):
            x_s = pool.tile([P, DH, w], bf, tag=tag, name=nm, bufs=2,
                            padded_shape=[P, DH, R])
            for h in range(0, DH, step):
                nc.sync.dma_start(x_s[:, h:h + step, :],
                                  src3[:, h:h + step, :])
            return x_s

        def proj_kt(dst, w_s, x_s, snd=None):
            nk = x_s.shape[-1]
            for oh in range(DH):
                pk = psA.tile([P, 512], f32, tag="mm", name="pk")
                for di in range(DH):
                    nc.tensor.matmul(pk[:, :nk], w_s[:, di, oh * P:(oh + 1) * P],
                                     x_s[:, di, :],
                                     start=(di == 0), stop=(di == DH - 1))
                nc.scalar.copy(dst[:, oh, :], pk[:, :nk])
                if snd is not None:
                    nc.sync.dma_start(snd[:, oh, :], dst[:, oh, :])

        def proj_v(dst, w_s, x_s, snd=None):
            nk = x_s.shape[-1]
            for rr in range(nk // P):
                for dvh in range(2):
                    pv = psA.tile([P, 512], f32, tag="mm", name="pv")
                    for di in range(DH):
                        nc.tensor.matmul(pv[:], x_s[:, di, rr * P:(rr + 1) * P],
                                         w_s[:, di, dvh * 512:(dvh + 1) * 512],
                                         start=(di == 0), stop=(di == DH - 1))
                    nc.vector.tensor_copy(dst[:, rr, dvh * 512:(dvh + 1) * 512],
                                          pv[:])
                if snd is not None:
                    nc.sync.dma_start(snd[:, rr, :], dst[:, rr, :])

        with tc.tile_pool(name="wp", bufs=1) as wp:
            # ---- K/V projection of own key block, then all-gather ----
            wk_s = load_w(wp, "k", "wk_s", step=1)
            xkv_s = load_x(wp, xkvt3, "xkv_s", w=OWN, step=2)
            kT_own = wp.tile([P, DH, OWN], bf, tag="kT_own")
            proj_kt(kT_own, wk_s, xkv_s, send_k)
            wv_s = load_w(wp, "v", "wv_s")
            v_own = wp.tile([P, OWN // P, D], bf, tag="v_own")
            proj_v(v_own, wv_s, xkv_s, send_v)

            nc.gpsimd.collective_compute(
                "AllGather", mybir.AluOpType.bypass,
                replica_groups=[list(range(NCORES))],
                ins=[send.opt()], outs=[gath.opt()])

            # ---- q projection (overlaps the all-gather) ----
            wq_s = load_w(wp, "q", "wq_s")
            xq_s = load_x(wp, xqt3, "xq_s")
            for oh in range(DH):
                pq = psA.tile([P, 512], f32, tag="mm", name="pq")
                for di in range(DH):
                    nc.tensor.matmul(pq[:], wq_s[:, di, oh * P:(oh + 1) * P],
                                     xq_s[:, di, :],
                                     start=(di == 0), stop=(di == DH - 1))
                nc.scalar.copy(qT[:, oh, :], pq[:])

            # ---- local recompute of early K/V chunks (fills the AG window) ----
            wk2 = load_w(wp, "k", "wk2")
            wv2 = load_w(wp, "v", "wv2")
            for rc in range(NLOC):
                xl_s = load_x(wp, xlt3[:, :, rc * 512:(rc + 1) * 512],
                              f"xl{rc}")
                proj_kt(kT_t[rc], wk2, xl_s)
                proj_v(v_t[rc], wv2, xl_s)

        # unpack gathered K/V sections into SBUF
        for s in range(NCORES):
            g_k = gath[s, 0].rearrange("(h p n) -> p h n", p=P, h=DH)
            g_v = gath[s, 1].rearrange("(n2 p d) -> p n2 d", p=P, n2=OWN // P)
            nc.gpsimd.dma_start(kTg[s][:], g_k[:])
            nc.gpsimd.dma_start(vg[s][:], g_v[:])

        def kt_sl(kj, dh):
            if kj < 4 * NLOC:
                return kT_t[kj // 4][:, dh, (kj % 4) * P:(kj % 4 + 1) * P]
            s = (P * kj - 512 * NLOC) // OWN
            col = P * kj - 512 * NLOC - OWN * s
            return kTg[s][:, dh, col:col + P]

        def v_sl(kj, dvh):
            if kj < 4 * NLOC:
                return v_t[kj // 4][:, kj % 4, dvh * 512:(dvh + 1) * 512]
            s = (P * kj - 512 * NLOC) // OWN
            rr = (P * kj - 512 * NLOC - OWN * s) // P
            return vg[s][:, rr, dvh * 512:(dvh + 1) * 512]

        # ---- attention ----
        with tc.tile_pool(name="at", bufs=4) as at, \
             tc.tile_pool(name="cs", bufs=1) as cs, \
             tc.tile_pool(name="dmp", bufs=4) as dmp:
            ones = cs.tile([P, 1], bf, tag="ones")
            nc.vector.memset(ones[:], 1.0)
            cm_s = cs.tile([P, 8, P], bf, tag="cm")
            nc.sync.dma_start(cm_s[:], cm3[:])

            for t in range(NT):
                nkj = 8 * (t + 1)           # 128-wide key chunks this block
                ng = nkj // 4               # psum groups of 4 chunks
                qsl = slice(t * P, (t + 1) * P)
                dmt = dmp.tile([P, 32, P], bf, tag="dm")
                half = max(nkj // 2, 4)
                for kc in range(0, nkj, half):
                    nc.scalar.dma_start(dmt[:, kc:kc + half, :],
                                        dm3[:, kc:kc + half, qsl])

                po = [psO.tile([P, 512], f32, tag="po", name=f"po{t}_{i}")
                      for i in range(2)]
                pd = psD.tile([P, 1], f32, tag="pd")

                for g in range(ng):
                    ps = psA.tile([P, 4, P], f32, tag="mm", name="ps")
                    for kj4 in range(4):
                        kj = g * 4 + kj4
                        for dh in range(DH):
                            nc.tensor.matmul(ps[:, kj4, :],
                                             kt_sl(kj, dh),
                                             qT[:, dh, qsl],
                                             start=(dh == 0), stop=(dh == DH - 1))
                    p = at.tile([P, 4, P], bf, tag="p")
                    nc.scalar.activation(p[:], ps[:],
                                         mybir.ActivationFunctionType.Exp,
                                         scale=1.0 / 32.0)
                    if g >= 2 * t:          # causal edge band: zero j > row
                        e = g - 2 * t
                        nc.vector.tensor_mul(p[:], p[:], cm_s[:, 4 * e:4 * e + 4, :])
                    pm = at.tile([P, 4, P], bf, tag="pm")
                    nc.vector.tensor_mul(pm[:], p[:], dmt[:, 4 * g:4 * g + 4, :])
                    for kj4 in range(4):
                        kj = g * 4 + kj4
                        nc.tensor.matmul(pd[:], p[:, kj4, :], ones[:],
                                         start=(kj == 0), stop=(kj == nkj - 1))
                        for dvh in range(2):
                            nc.tensor.matmul(po[dvh][:], pm[:, kj4, :],
                                             v_sl(kj, dvh),
                                             start=(kj == 0), stop=(kj == nkj - 1))

                den = at.tile([P, 1], f32, tag="den")
                nc.vector.tensor_scalar_mul(den[:], pd[:], KEEP)
                rec = at.tile([P, 1], f32, tag="rec")
                nc.vector.reciprocal(rec[:], den[:])
                for dvh in range(2):
                    ob = at.tile([P, 512], f32, tag="ob")
                    nc.vector.tensor_scalar_mul(ob[:], po[dvh][:], rec[:])
                    nc.sync.dma_start(out[qsl, dvh * 512:(dvh + 1) * 512], ob[:])


_CACHE = {}


def _get_nc():
    if "nc" not in _CACHE:
        nc = bacc.Bacc("TRN2", target_bir_lowering=False, debug=False,
                       num_devices=NCORES)
        aps = (
            nc.dram_tensor("xqt", [D, R], bf, kind="ExternalInput").ap(),
            nc.dram_tensor("xkvt", [D, OWN], bf, kind="ExternalInput").ap(),
            nc.dram_tensor("xlt", [D, 512 * NLOC], bf,
                           kind="ExternalInput").ap(),
            nc.dram_tensor("wqt", [D, D], bf, kind="ExternalInput").ap(),
            nc.dram_tensor("wkt", [D, D], bf, kind="ExternalInput").ap(),
            nc.dram_tensor("wvt", [D, D], bf, kind="ExternalInput").ap(),
            nc.dram_tensor("cm", [1024, P], bf, kind="ExternalInput").ap(),
            nc.dram_tensor("dm", [N, R], bf, kind="ExternalInput").ap(),
            nc.dram_tensor("out", [R, D], f32, kind="ExternalOutput").ap(),
        )
        with tile.TileContext(nc) as tc:
            _body(nc, tc, aps)
        nc.compile()
        _CACHE["nc"] = nc
    return _CACHE["nc"]


def _dropout_keep():
    # Must reproduce the reference's mask bit-exactly. The reference calls
    # jax.random.bernoulli on whatever jax backend is the process default
    # (different backends produce different threefry layouts), so mirror the
    # call verbatim with no device pinning.
    if "keep" not in _CACHE:
        import jax
        keep = np.asarray(
            jax.random.bernoulli(jax.random.key(42), KEEP, (N, N)))
        _CACHE["keep"] = keep
    return _CACHE["keep"]


def _c(a):
    return np.ascontiguousarray(a)


def kernel(x, wq, wk, wv, **_spmd_kwargs):
    x = np.asarray(x, np.float32)
    wqt = _c(np.asarray(wq, np.float32).astype(bfnp).T)
    wkt = _c(np.asarray(wk, np.float32).astype(bfnp).T)
    wvt = _c(np.asarray(wv, np.float32).astype(bfnp).T)
    xb = x.astype(bfnp)
    xlt = _c(xb[:512 * NLOC].T)
    keep = _dropout_keep()

    jj = np.arange(1024)[:, None]
    ii = np.arange(P)[None, :]

    in_maps = []
    for c in range(NCORES):
        cm = ((jj <= 8 * ii + c).astype(np.float32)).astype(bfnp)
        dm = _c(keep[c::NCORES, :].T.astype(bfnp))
        in_maps.append({
            "xqt": _c(xb[c::NCORES].T),
            "xkvt": _c(xb[512 * NLOC + c * OWN:512 * NLOC + (c + 1) * OWN].T),
            "xlt": xlt,
            "wqt": wqt, "wkt": wkt, "wvt": wvt,
            "cm": _c(cm), "dm": dm,
        })

    nc = _get_nc()
    res = run_bass_kernel_spmd(nc, in_maps, core_ids=list(range(NCORES)),
                               **_spmd_kwargs)
    out = np.empty((N, D), np.float32)
    for c in range(NCORES):
        out[c::NCORES] = res.results[c]["out"]
    if _spmd_kwargs:
        kernel.last_result = res
    return out


# revision 23
# speedup vs baseline: 1.3644x; 1.0092x over previous
"""Causal attention (N=4096, D=1024) + dropout, distributed over 8 TRN2 NeuronCores.

Sharding: sequence-parallel over Q rows, strided assignment (core c owns rows
c::8) so every core has an identical causal-work profile (SPMD: one program).
K/V projections are sharded: core c computes K/V for contiguous key block
[512c, 512c+512), then one AllGather shares all K/V shards with every core.

Per-core device program (all matmuls bf16 inputs, f32 PSUM accumulation):
  qT = wqT.T @ xqT            [d_out, 512]   (feature-major)
  kT_own = wkT.T @ xkvT       [d_out, 512]   (own key block)
  v_own  = xkvT.T @ wvT       [512, d_out]
  AllGather(kT_own | v_own)  -> kT [d_out, 4096], v [4096, d_out]
  per qi-block t (128 local rows = global span 1024t..1024(t+1)):
    sT[kj,qi] = kT.T @ qT     only kj < 1024(t+1)  (causal block skipping)
    p = exp(sT/32) * causal_edge_mask
    denom[qi] = ones-matmul over p
    pm = p * dropout_mask
    out[qi,:] = (pm.T @ v) * 1/(0.9*denom)
"""

import numpy as np
import ml_dtypes

import concourse.bass as bass
import concourse.mybir as mybir
import concourse.tile as tile
from concourse import bacc
from concourse.bass_utils import run_bass_kernel_spmd

N, D, P, NCORES = 4096, 1024, 128, 8
R = N // NCORES          # 512 q rows per core
DH = D // P              # 8 chunks of the feature dim
NT = R // P              # 4 qi-blocks per core
KEEP = 0.9

bf = mybir.dt.bfloat16
f32 = mybir.dt.float32
bfnp = ml_dtypes.bfloat16


NLOC = 2                 # 512-key chunks recomputed locally during the all-gather
SHK = N - 512 * NLOC     # keys shared via all-gather
OWN = SHK // NCORES      # 384 keys projected per core


def _body(nc, tc, aps):
    xqt, xkvt, xlt, wqt, wkt, wvt, cm, dm, out = aps
    xqt3 = xqt.rearrange("(h p) r -> p h r", p=P)
    xkvt3 = xkvt.rearrange("(h p) r -> p h r", p=P)
    xlt3 = xlt.rearrange("(h p) r -> p h r", p=P)
    w3 = {k: v.rearrange("(h p) o -> p h o", p=P) for k, v in
          dict(q=wqt, k=wkt, v=wvt).items()}
    cm3 = cm.rearrange("(e p) i -> p e i", p=P)
    dm3 = dm.rearrange("(kc p) q -> p kc q", p=P)

    import contextlib
    with contextlib.ExitStack() as ctx:
        big = ctx.enter_context(tc.tile_pool(name="big", bufs=1))
        dram = ctx.enter_context(tc.tile_pool(name="dram", bufs=1, space="DRAM"))
        psA = ctx.enter_context(tc.tile_pool(name="psA", bufs=4, space="PSUM"))
        psO = ctx.enter_context(tc.tile_pool(name="psO", bufs=2, space="PSUM"))
        psD = ctx.enter_context(tc.tile_pool(name="psD", bufs=2, space="PSUM"))

        # persistent SBUF: qT, local K/V chunks (keys < 1024), gathered sections
        qT = big.tile([P, DH, R], bf, tag="qT")
        kT_t = [big.tile([P, DH, 512], bf, tag=f"kT{rc}", name=f"kT{rc}")
                for rc in range(NLOC)]
        v_t = [big.tile([P, 4, D], bf, tag=f"v{rc}", name=f"v{rc}")
               for rc in range(NLOC)]
        kTg = [big.tile([P, DH, OWN], bf, tag=f"kTg{s}", name=f"kTg{s}")
               for s in range(NCORES)]
        vg = [big.tile([P, OWN // P, D], bf, tag=f"vg{s}", name=f"vg{s}")
              for s in range(NCORES)]

        # DRAM bounce buffers for the K/V all-gather
        send = dram.tile([2, D * OWN], bf, name="send")
        gath = dram.tile([NCORES, 2, D * OWN], bf, addr_space="Shared",
                         name="gath")
        send_k = send[0].rearrange("(h p n) -> p h n", p=P, h=DH)
        send_v = send[1].rearrange("(n2 p d) -> p n2 d", p=P, n2=OWN // P)

        def load_w(pool, which, nm, step=4):
            w_s = pool.tile([P, DH, D], bf, tag="w", name=nm, bufs=2)
            for h in range(0, DH, step):
                nc.sync.dma_start(w_s[:, h:h + step, :],
                                  w3[which][:, h:h + step, :])
            return w_s

        def load_x(pool, src3, nm, tag="xs", w=R, step=4):
            x_s = pool.tile([P, DH, w], bf, tag=tag, name=nm, bufs=2,
                            padded_shape=[P, DH, R])
            for h in range(0, DH, step):
                nc.sync.dma_start(x_s[:, h:h + step, :],
                                  src3[:, h:h + step, :])
            return x_s

        def proj_kt(dst, w_s, x_s, snd=None):
            nk = x_s.shape[-1]
            for oh in range(DH):
                pk = psA.tile([P, 512], f32, tag="mm", name="pk")
                for di in range(DH):
                    nc.tensor.matmul(pk[:, :nk], w_s[:, di, oh * P:(oh + 1) * P],
                                     x_s[:, di, :],
                                     start=(di == 0), stop=(di == DH - 1))
                nc.scalar.copy(dst[:, oh, :], pk[:, :nk])
                if snd is not None:
                    nc.sync.dma_start(snd[:, oh, :], dst[:, oh, :])

        def proj_v(dst, w_s, x_s, snd=None):
            nk = x_s.shape[-1]
            for rr in range(nk // P):
                for dvh in range(2):
                    pv = psA.tile([P, 512], f32, tag="mm", name="pv")
                    for di in range(DH):
                        nc.tensor.matmul(pv[:], x_s[:, di, rr * P:(rr + 1) * P],
                                         w_s[:, di, dvh * 512:(dvh + 1) * 512],
                                         start=(di == 0), stop=(di == DH - 1))
                    nc.vector.tensor_copy(dst[:, rr, dvh * 512:(dvh + 1) * 512],
                                          pv[:])
                if snd is not None:
                    nc.sync.dma_start(snd[:, rr, :], dst[:, rr, :])

        with tc.tile_pool(name="wp", bufs=1) as wp:
            # ---- K/V projection of own key block, then all-gather ----
            wk_s = load_w(wp, "k", "wk_s", step=1)
            xkv_s = load_x(wp, xkvt3, "xkv_s", w=OWN, step=2)
            kT_own = wp.tile([P, DH, OWN], bf, tag="kT_own")
            proj_kt(kT_own, wk_s, xkv_s, send_k)
            wv_s = load_w(wp, "v", "wv_s")
            v_own = wp.tile([P, OWN // P, D], bf, tag="v_own")
            proj_v(v_own, wv_s, xkv_s, send_v)

            nc.gpsimd.collective_compute(
                "AllGather", mybir.AluOpType.bypass,
                replica_groups=[list(range(NCORES))],
                ins=[send.opt()], outs=[gath.opt()])

            # ---- q projection (overlaps the all-gather) ----
            wq_s = load_w(wp, "q", "wq_s")
            xq_s = load_x(wp, xqt3, "xq_s")
            for oh in range(DH):
                pq = psA.tile([P, 512], f32, tag="mm", name="pq")
                for di in range(DH):
                    nc.tensor.matmul(pq[:], wq_s[:, di, oh * P:(oh + 1) * P],
                                     xq_s[:, di, :],
                                     start=(di == 0), stop=(di == DH - 1))
                nc.scalar.copy(qT[:, oh, :], pq[:])

            # ---- local recompute of early K/V chunks (fills the AG window) ----
            wk2 = load_w(wp, "k", "wk2")
            wv2 = load_w(wp, "v", "wv2")
            for rc in range(NLOC):
                xl_s = load_x(wp, xlt3[:, :, rc * 512:(rc + 1) * 512],
                              f"xl{rc}")
                proj_kt(kT_t[rc], wk2, xl_s)
                proj_v(v_t[rc], wv2, xl_s)

        # unpack gathered K/V sections into SBUF; per pair of sections pull
        # both K sections before the V sections (scores consume K first)
        for s0 in range(0, NCORES, 2):
            for s in (s0, s0 + 1):
                g_k = gath[s, 0].rearrange("(h p n) -> p h n", p=P, h=DH)
                nc.gpsimd.dma_start(kTg[s][:], g_k[:])
            for s in (s0, s0 + 1):
                g_v = gath[s, 1].rearrange("(n2 p d) -> p n2 d", p=P, n2=OWN // P)
                nc.gpsimd.dma_start(vg[s][:], g_v[:])

        def kt_sl(kj, dh):
            if kj < 4 * NLOC:
                return kT_t[kj // 4][:, dh, (kj % 4) * P:(kj % 4 + 1) * P]
            s = (P * kj - 512 * NLOC) // OWN
            col = P * kj - 512 * NLOC - OWN * s
            return kTg[s][:, dh, col:col + P]

        def v_sl(kj, dvh):
            if kj < 4 * NLOC:
                return v_t[kj // 4][:, kj % 4, dvh * 512:(dvh + 1) * 512]
            s = (P * kj - 512 * NLOC) // OWN
            rr = (P * kj - 512 * NLOC - OWN * s) // P
            return vg[s][:, rr, dvh * 512:(dvh + 1) * 512]

        # ---- attention ----
        with tc.tile_pool(name="at", bufs=4) as at, \
             tc.tile_pool(name="cs", bufs=1) as cs, \
             tc.tile_pool(name="dmp", bufs=4) as dmp:
            ones = cs.tile([P, 1], bf, tag="ones")
            nc.vector.memset(ones[:], 1.0)
            cm_s = cs.tile([P, 8, P], bf, tag="cm")
            nc.sync.dma_start(cm_s[:], cm3[:])

            for t in range(NT):
                nkj = 8 * (t + 1)           # 128-wide key chunks this block
                ng = nkj // 4               # psum groups of 4 chunks
                qsl = slice(t * P, (t + 1) * P)
                dmt = dmp.tile([P, 32, P], bf, tag="dm")
                half = max(nkj // 2, 4)
                for kc in range(0, nkj, half):
                    nc.scalar.dma_start(dmt[:, kc:kc + half, :],
                                        dm3[:, kc:kc + half, qsl])

                po = [psO.tile([P, 512], f32, tag="po", name=f"po{t}_{i}")
                      for i in range(2)]
                pd = psD.tile([P, 1], f32, tag="pd")

                for g in range(ng):
                    ps = psA.tile([P, 4, P], f32, tag="mm", name="ps")
                    for kj4 in range(4):
                        kj = g * 4 + kj4
                        for dh in range(DH):
                            nc.tensor.matmul(ps[:, kj4, :],
                                             kt_sl(kj, dh),
                                             qT[:, dh, qsl],
                                             start=(dh == 0), stop=(dh == DH - 1))
                    p = at.tile([P, 4, P], bf, tag="p")
                    nc.scalar.activation(p[:], ps[:],
                                         mybir.ActivationFunctionType.Exp,
                                         scale=1.0 / 32.0)
                    if g >= 2 * t:          # causal edge band: zero j > row
                        e = g - 2 * t
                        nc.vector.tensor_mul(p[:], p[:], cm_s[:, 4 * e:4 * e + 4, :])
                    pm = at.tile([P, 4, P], bf, tag="pm")
                    nc.vector.tensor_mul(pm[:], p[:], dmt[:, 4 * g:4 * g + 4, :])
                    for kj4 in range(4):
                        kj = g * 4 + kj4
                        nc.tensor.matmul(pd[:], p[:, kj4, :], ones[:],
                                         start=(kj == 0), stop=(kj == nkj - 1))
                        for dvh in range(2):
                            nc.tensor.matmul(po[dvh][:], pm[:, kj4, :],
                                             v_sl(kj, dvh),
                                             start=(kj == 0), stop=(kj == nkj - 1))

                den = at.tile([P, 1], f32, tag="den")
                nc.vector.tensor_scalar_mul(den[:], pd[:], KEEP)
                rec = at.tile([P, 1], f32, tag="rec")
                nc.vector.reciprocal(rec[:], den[:])
                for dvh in range(2):
                    ob = at.tile([P, 512], f32, tag="ob")
                    nc.vector.tensor_scalar_mul(ob[:], po[dvh][:], rec[:])
                    nc.sync.dma_start(out[qsl, dvh * 512:(dvh + 1) * 512], ob[:])


_CACHE = {}


def _get_nc():
    if "nc" not in _CACHE:
        nc = bacc.Bacc("TRN2", target_bir_lowering=False, debug=False,
                       num_devices=NCORES)
        aps = (
            nc.dram_tensor("xqt", [D, R], bf, kind="ExternalInput").ap(),
            nc.dram_tensor("xkvt", [D, OWN], bf, kind="ExternalInput").ap(),
            nc.dram_tensor("xlt", [D, 512 * NLOC], bf,
                           kind="ExternalInput").ap(),
            nc.dram_tensor("wqt", [D, D], bf, kind="ExternalInput").ap(),
            nc.dram_tensor("wkt", [D, D], bf, kind="ExternalInput").ap(),
            nc.dram_tensor("wvt", [D, D], bf, kind="ExternalInput").ap(),
            nc.dram_tensor("cm", [1024, P], bf, kind="ExternalInput").ap(),
            nc.dram_tensor("dm", [N, R], bf, kind="ExternalInput").ap(),
            nc.dram_tensor("out", [R, D], f32, kind="ExternalOutput").ap(),
        )
        with tile.TileContext(nc) as tc:
            _body(nc, tc, aps)
        nc.compile()
        _CACHE["nc"] = nc
    return _CACHE["nc"]


def _dropout_keep():
    # Must reproduce the reference's mask bit-exactly. The reference calls
    # jax.random.bernoulli on whatever jax backend is the process default
    # (different backends produce different threefry layouts), so mirror the
    # call verbatim with no device pinning.
    if "keep" not in _CACHE:
        import jax
        keep = np.asarray(
            jax.random.bernoulli(jax.random.key(42), KEEP, (N, N)))
        _CACHE["keep"] = keep
    return _CACHE["keep"]


def _c(a):
    return np.ascontiguousarray(a)


def kernel(x, wq, wk, wv, **_spmd_kwargs):
    x = np.asarray(x, np.float32)
    wqt = _c(np.asarray(wq, np.float32).astype(bfnp).T)
    wkt = _c(np.asarray(wk, np.float32).astype(bfnp).T)
    wvt = _c(np.asarray(wv, np.float32).astype(bfnp).T)
    xb = x.astype(bfnp)
    xlt = _c(xb[:512 * NLOC].T)
    keep = _dropout_keep()

    jj = np.arange(1024)[:, None]
    ii = np.arange(P)[None, :]

    in_maps = []
    for c in range(NCORES):
        cm = ((jj <= 8 * ii + c).astype(np.float32)).astype(bfnp)
        dm = _c(keep[c::NCORES, :].T.astype(bfnp))
        in_maps.append({
            "xqt": _c(xb[c::NCORES].T),
            "xkvt": _c(xb[512 * NLOC + c * OWN:512 * NLOC + (c + 1) * OWN].T),
            "xlt": xlt,
            "wqt": wqt, "wkt": wkt, "wvt": wvt,
            "cm": _c(cm), "dm": dm,
        })

    nc = _get_nc()
    res = run_bass_kernel_spmd(nc, in_maps, core_ids=list(range(NCORES)),
                               **_spmd_kwargs)
    out = np.empty((N, D), np.float32)
    for c in range(NCORES):
        out[c::NCORES] = res.results[c]["out"]
    if _spmd_kwargs:
        kernel.last_result = res
    return out
